# revision 13
# baseline (speedup 1.0000x reference)
"""AxialAttention (MSA row attention) on 8 Trainium2 NeuronCores.

Sharding: data parallel over MSA rows r=128 (16 rows/core); the edge-bias
precompute is sharded over the edge i dim (32 i-rows/core) in kernel 1,
gathered on host, replicated into kernel 2 (attention).

v2 design notes (vs v1 baseline at 662us):
  - all matmuls in bf16 (v1 ran softmax-sum + attn@V matmuls as fp32 at
    4 cycles/row; bf16 is 1 cycle/row).
  - head layout: 2 blocks x 4 heads at partition offsets {0,32,64,96};
    SLOTS = 256 (v1: 3 blocks, 384).
  - denominator via ones-matmul replicated per head (32 rows), attn@V and
    denominator accumulate in one PSUM bank per block: av | den.
  - dots for a head PAIR share one PSUM tile [128, 512] -> one exp
    activation per pair (halves ACT instruction count).
  - ACT keeps ONE table resident (exp_and_others: Exp/Identity/Tanh):
    sigmoid(x) = 0.5*tanh(x/2)+0.5, LN rstd sqrt is batched into a single
    prologue op before the first exp (2 table loads total; v1 had 35).
  - LN stats in a prologue over SBUF-resident x; batched smalls.
  - few large DMAs instead of many small ones (HWDGE is 625ns/DMA).
"""

import sys
import numpy as np

sys.path.insert(0, "/opt/trn_rl_repo")

import concourse.bacc as bacc
import concourse.tile as tile
import concourse.bass as bass
from concourse import mybir
from concourse import bass_utils

F32 = mybir.dt.float32
BF16 = mybir.dt.bfloat16
U8 = mybir.dt.uint8
AF = mybir.ActivationFunctionType
MUL = mybir.AluOpType.mult
ADD = mybir.AluOpType.add

NC = 8
B, R, W, DN = 1, 128, 256, 256
DE, H, DH = 128, 8, 32
RPC = R // NC    # rows per core = 16
IPC = W // NC    # edge i-rows per core = 32
NEG = -1.0e38
EPS = 1e-5

NB = 3                  # head blocks (3/3/2 heads); base partition must be
SLOTS = NB * 128        # in {0,32,64}, so only 3 heads fit per 128-block
HB_ROWS = [96, 96, 64]  # used partitions per block
HB_HEADS = [3, 3, 2]


def _head_slot(h):
    return (h // 3) * 128 + 32 * (h % 3)


def _expand_cols(Wm):
    D = Wm.shape[0]
    out = np.zeros((D, SLOTS), Wm.dtype)
    for h in range(H):
        out[:, _head_slot(h):_head_slot(h) + DH] = Wm[:, h * DH:(h + 1) * DH]
    return out


def _expand_rows(Wm):
    D = Wm.shape[1]
    out = np.zeros((SLOTS, D), Wm.dtype)
    for h in range(H):
        out[_head_slot(h):_head_slot(h) + DH, :] = Wm[h * DH:(h + 1) * DH, :]
    return out


def _bf16(a):
    import ml_dtypes
    return np.asarray(a).astype(ml_dtypes.bfloat16)


# ---------------------------------------------------------------- kernel 1
def _build_bias_nc():
    """Per core: edges slice [IPC*W, DE] -> bias part [H, IPC*W]."""
    nc = bacc.Bacc("TRN2", target_bir_lowering=False, debug=False,
                   num_devices=NC)
    P = 128
    TOK = IPC * W          # 8192
    NT = TOK // P          # 64 token tiles
    e_d = nc.dram_tensor("e", [TOK, DE], F32, kind="ExternalInput").ap()
    we_d = nc.dram_tensor("we", [DE, H], BF16, kind="ExternalInput").ap()
    id_d = nc.dram_tensor("idm", [P, P], BF16, kind="ExternalInput").ap()
    o_d = nc.dram_tensor("o", [H, TOK], F32, kind="ExternalOutput").ap()

    with tile.TileContext(nc) as tc:
        with tc.tile_pool(name="cst", bufs=1) as cst, \
             tc.tile_pool(name="work", bufs=6) as work, \
             tc.tile_pool(name="tp", bufs=3) as tp, \
             tc.tile_pool(name="pst", bufs=3, space="PSUM") as pst, \
             tc.tile_pool(name="psb", bufs=2, space="PSUM") as psb:
            ident = cst.tile([P, P], BF16)
            nc.sync.dma_start(out=ident, in_=id_d)
            we_sb = cst.tile([DE, H], BF16)
            nc.sync.dma_start(out=we_sb, in_=we_d)
            eps_sb = cst.tile([P, 1], F32)
            nc.vector.memset(eps_sb, EPS)

            e_all = cst.tile([P, NT, DE], F32, name="e_all")
            for i in range(16):
                nc.sync.dma_start(
                    out=e_all[:, i * 4:(i + 1) * 4, :],
                    in_=bass.AP(tensor=e_d.tensor, offset=i * 4 * P * DE,
                                ap=[[DE, P], [P * DE, 4], [1, DE]]))

            mv = cst.tile([P, 2, NT], F32, name="mv")
            ost = cst.tile([H, TOK], F32, name="ost")

            for rep in range(REPEAT):
                for t in range(NT):
                    st = work.tile([P, 6], F32, tag="st")
                    nc.vector.bn_stats(out=st, in_=e_all[:, t, :])
                    nc.vector.bn_aggr(out=mv[:, :, t], in_=st)
                # batched smalls
                sd = work.tile([P, NT], F32, tag="sd")
                nc.scalar.activation(sd, mv[:, 1, :], AF.Sqrt,
                                     bias=eps_sb[:])
                rstd = work.tile([P, NT], F32, tag="rs")
                nc.vector.reciprocal(rstd, sd)
                nmr = work.tile([P, NT], F32, tag="nm")
                nc.vector.scalar_tensor_tensor(
                    out=nmr, in0=mv[:, 0, :], scalar=-1.0, in1=rstd,
                    op0=MUL, op1=MUL)

                for g in range(NT // 4):
                    pt = pst.tile([DE, 4 * P], BF16, tag="pt")
                    for u in range(4):
                        t = g * 4 + u
                        en = work.tile([P, DE], BF16, tag="en")
                        nc.scalar.activation(en, e_all[:, t, :], AF.Identity,
                                             bias=nmr[:, t:t + 1],
                                             scale=rstd[:, t:t + 1])
                        nc.tensor.matmul(pt[:, u * P:(u + 1) * P],
                                         en[:], ident[:],
                                         is_transpose=True,
                                         start=(u == 0), stop=(u == 3))
                    enT = tp.tile([DE, 4 * P], BF16, tag="enT")
                    if g % 2 == 0:
                        nc.vector.tensor_copy(out=enT, in_=pt)
                    else:
                        nc.scalar.copy(out=enT, in_=pt)
                    ob = psb.tile([H, 4 * P], F32, tag="ob")
                    nc.tensor.matmul(ob[:], we_sb[:], enT[:],
                                     start=True, stop=True)
                    if g % 2 == 0:
                        nc.scalar.copy(out=ost[:, g * 4 * P:(g + 1) * 4 * P],
                                       in_=ob)
                    else:
                        nc.vector.tensor_copy(
                            out=ost[:, g * 4 * P:(g + 1) * 4 * P], in_=ob)
                for i in range(2):
                    nc.sync.dma_start(
                        out=o_d[:, i * (TOK // 2):(i + 1) * (TOK // 2)],
                        in_=ost[:, i * (TOK // 2):(i + 1) * (TOK // 2)])
    nc.compile()
    return nc


# ---------------------------------------------------------------- kernel 2
def _build_attn_nc():
    nc = bacc.Bacc("TRN2", target_bir_lowering=False, debug=False,
                   num_devices=NC)
    P = 128
    TOK = RPC * W          # 4096 tokens per core
    NT = TOK // P          # 32 token tiles
    CH = 512               # tokens per chunk (2 rows)
    NCH = TOK // CH        # 8 chunks

    x_d = nc.dram_tensor("x", [TOK, DN], F32, kind="ExternalInput").ap()
    wq_d = nc.dram_tensor("wq", [DN, SLOTS], BF16, kind="ExternalInput").ap()
    wk_d = nc.dram_tensor("wk", [DN, SLOTS], BF16, kind="ExternalInput").ap()
    wv_d = nc.dram_tensor("wv", [DN, SLOTS], BF16, kind="ExternalInput").ap()
    wg_d = nc.dram_tensor("wg", [DN, SLOTS], BF16, kind="ExternalInput").ap()
    wo_d = nc.dram_tensor("wo", [SLOTS, DN], BF16, kind="ExternalInput").ap()
    bg_d = nc.dram_tensor("bg", [P, NB], F32, kind="ExternalInput").ap()
    bo_d = nc.dram_tensor("bo", [1, DN], BF16, kind="ExternalInput").ap()
    bt_d = nc.dram_tensor("bt", [P, H, 2, W], BF16, kind="ExternalInput").ap()
    id_d = nc.dram_tensor("idm", [P, P], BF16, kind="ExternalInput").ap()
    ngj_d = nc.dram_tensor("ngj", [P, RPC * 2], F32,
                           kind="ExternalInput").ap()
    invm_d = nc.dram_tensor("invm", [RPC, W], U8, kind="ExternalInput").ap()
    o_d = nc.dram_tensor("o", [TOK, DN], F32, kind="ExternalOutput").ap()

    with tile.TileContext(nc, trace_sim=SIM_TRACE) as tc:
        from contextlib import ExitStack
        with ExitStack() as ctx:
            cst = ctx.enter_context(tc.tile_pool(name="cst", bufs=1))
            work = ctx.enter_context(tc.tile_pool(name="work", bufs=6))
            chw = ctx.enter_context(tc.tile_pool(name="chw", bufs=2))
            expp = ctx.enter_context(tc.tile_pool(name="expp", bufs=4))
            rowp = ctx.enter_context(tc.tile_pool(name="rowp", bufs=3))
            ps = ctx.enter_context(
                tc.tile_pool(name="ps", bufs=1, space="PSUM"))

            ident = cst.tile([P, P], BF16)
            nc.sync.dma_start(out=ident, in_=id_d)
            ones_blk = cst.tile([P, DH], BF16)
            nc.vector.memset(ones_blk, 1.0)
            ones_row = cst.tile([1, P], BF16)
            nc.vector.memset(ones_row, 1.0)
            wns = cst.tile([P, 1], BF16)
            nc.vector.memset(wns, 1.0 / W)
            eps_sb = cst.tile([P, 1], F32)
            nc.vector.memset(eps_sb, EPS)

            def load_w(d, shape, nm, dt=BF16):
                t = cst.tile(shape, dt, tag=nm, name=nm)
                nc.sync.dma_start(out=t, in_=d)
                return t

            wq = [load_w(wq_d[kt * P:(kt + 1) * P, :], [P, SLOTS], f"wq{kt}")
                  for kt in range(2)]
            wk = [load_w(wk_d[kt * P:(kt + 1) * P, :], [P, SLOTS], f"wk{kt}")
                  for kt in range(2)]
            wv = [load_w(wv_d[kt * P:(kt + 1) * P, :], [P, SLOTS], f"wv{kt}")
                  for kt in range(2)]
            wg = [load_w(wg_d[kt * P:(kt + 1) * P, :], [P, SLOTS], f"wg{kt}")
                  for kt in range(2)]
            wo = [load_w(wo_d[b * P:b * P + HB_ROWS[b], :],
                         [HB_ROWS[b], DN], f"wo{b}") for b in range(NB)]
            bg = load_w(bg_d, [P, NB], "bgt", F32)
            bo = load_w(bo_d, [1, DN], "bot")
            bt_sb = load_w(bt_d, [P, H * 2 * W], "btt")

            def bt_ap(h, jt):
                # [128, W] slice for head h, j-tile jt
                return bt_sb[:, (h * 2 + jt) * W:(h * 2 + jt + 1) * W]

            ngj = load_w(ngj_d, [P, RPC * 2], "ngjt", F32)

            x_all = cst.tile([P, NT, DN], F32, name="x_all")
            for i in range(8):
                nc.sync.dma_start(
                    out=x_all[:, i * 4:(i + 1) * 4, :],
                    in_=bass.AP(tensor=x_d.tensor, offset=i * 4 * P * DN,
                                ap=[[DN, P], [P * DN, 4], [1, DN]]))

            mv = cst.tile([P, 2, NT], F32, name="mv")
            xnT = [cst.tile([P, TOK], BF16, name=f"xnT{kt}")
                   for kt in range(2)]

            for rep in range(REPEAT):
                # ---- P1: LN stats (batched smalls; single Sqrt before exp)
                for t in range(NT):
                    st = work.tile([P, 6], F32, tag="st")
                    nc.vector.bn_stats(out=st, in_=x_all[:, t, :])
                    nc.vector.bn_aggr(out=mv[:, :, t], in_=st)
                sd = work.tile([P, NT], F32, tag="sd")
                nc.scalar.activation(sd, mv[:, 1, :], AF.Sqrt, bias=eps_sb[:])
                rstd = work.tile([P, NT], F32, tag="rs")
                nc.vector.reciprocal(rstd, sd)
                nmr = work.tile([P, NT], F32, tag="nm")
                nc.vector.scalar_tensor_tensor(
                    out=nmr, in0=mv[:, 0, :], scalar=-1.0, in1=rstd,
                    op0=MUL, op1=MUL)

                # ---- P2: xn + transpose -> xnT [2][128, TOK] bf16
                for g in range(NT // 4):
                    pt = [ps.tile([P, 4 * P], BF16, tag="tr", bufs=1,
                                  name=f"pt{g}_{kt}")
                          for kt in range(2)]
                    for u in range(4):
                        t = g * 4 + u
                        xn = work.tile([P, DN], BF16, tag="xn")
                        nc.scalar.activation(xn, x_all[:, t, :], AF.Identity,
                                             bias=nmr[:, t:t + 1],
                                             scale=rstd[:, t:t + 1])
                        for kt in range(2):
                            nc.tensor.matmul(pt[kt][:, u * P:(u + 1) * P],
                                             xn[:, kt * P:(kt + 1) * P],
                                             ident[:], is_transpose=True,
                                             start=(u == 0), stop=(u == 3))
                    for kt in range(2):
                        dst = xnT[kt][:, g * 4 * P:(g + 1) * 4 * P]
                        if (g + kt) % 2 == 0:
                            nc.vector.tensor_copy(out=dst, in_=pt[kt])
                        else:
                            nc.scalar.copy(out=dst, in_=pt[kt])

                # ---- P3: chunk loop
                for ch in range(NCH):
                    c0 = ch * CH
                    q_sb, k_sb, sig_sb, v_sb = [], [], [], []
                    for b in range(NB):
                        pp = ps.tile([P, CH], F32, tag="big", bufs=4, name="pp")
                        for kt in range(2):
                            nc.tensor.matmul(
                                pp[:], wq[kt][:, b * P:(b + 1) * P],
                                xnT[kt][:, c0:c0 + CH],
                                start=(kt == 0), stop=(kt == 1))
                        t = chw.tile([P, CH], BF16, tag=f"q{b}")
                        nc.vector.tensor_copy(out=t, in_=pp)
                        q_sb.append(t)
                    for b in range(NB):
                        pp = ps.tile([P, CH], F32, tag="big", bufs=4, name="pp")
                        for kt in range(2):
                            nc.tensor.matmul(
                                pp[:], wk[kt][:, b * P:(b + 1) * P],
                                xnT[kt][:, c0:c0 + CH],
                                start=(kt == 0), stop=(kt == 1))
                        t = chw.tile([P, CH], BF16, tag=f"k{b}")
                        nc.scalar.copy(out=t, in_=pp)
                        k_sb.append(t)
                    for b in range(NB):
                        pp = ps.tile([P, CH], F32, tag="big", bufs=4, name="pp")
                        for kt in range(2):
                            nc.tensor.matmul(
                                pp[:], wg[kt][:, b * P:(b + 1) * P],
                                xnT[kt][:, c0:c0 + CH],
                                start=(kt == 0), stop=(kt == 1))
                        # sigmoid(z) = 0.5*tanh(z/2) + 0.5 (bg/2 folded in)
                        tnh = work.tile([P, CH], BF16, tag=f"tn{b}")
                        nc.scalar.activation(tnh, pp, AF.Tanh,
                                             bias=bg[:, b:b + 1], scale=0.5)
                        sig = chw.tile([P, CH], BF16, tag=f"s{b}")
                        nc.gpsimd.tensor_scalar(out=sig, in0=tnh,
                                                scalar1=0.5, scalar2=0.5,
                                                op0=MUL, op1=ADD)
                        sig_sb.append(sig)
                    for tb in range(4):
                        pp = ps.tile([P, SLOTS], F32, tag="big", bufs=4, name="pp")
                        for kt in range(2):
                            nc.tensor.matmul(
                                pp[:], xnT[kt][:, c0 + tb * P:
                                               c0 + (tb + 1) * P],
                                wv[kt][:], start=(kt == 0), stop=(kt == 1))
                        t = chw.tile([P, SLOTS], BF16, tag=f"v{tb}")
                        if tb % 2 == 0:
                            nc.vector.tensor_copy(out=t, in_=pp)
                        else:
                            nc.scalar.copy(out=t, in_=pp)
                        v_sb.append(t)

                    # vbar for both rows of the chunk: one PSUM group
                    vbp = ps.tile([P, NB * 2], F32, tag="vb", bufs=1,
                                  name=f"vb{ch}")
                    nmm = 0
                    for b in range(NB):
                        for rl2 in range(2):
                            for jt in range(2):
                                nc.tensor.matmul(
                                    vbp[:, b * 2 + rl2:b * 2 + rl2 + 1],
                                    v_sb[2 * rl2 + jt][:, b * P:(b + 1) * P],
                                    wns[:], start=(nmm == 0),
                                    stop=(nmm == NB * 4 - 1),
                                    skip_group_check=True)
                                nmm += 1
                    vbar_sb = chw.tile([P, NB * 2], F32, tag="vbar")
                    nc.vector.tensor_copy(out=vbar_sb, in_=vbp)

                    # ---- per-row attention
                    for rl in range(2):
                        r = ch * 2 + rl
                        invm_b = rowp.tile([P, W], U8, tag="invm")
                        nc.sync.dma_start(
                            out=invm_b,
                            in_=bass.AP(tensor=invm_d.tensor, offset=r * W,
                                        ap=[[0, P], [1, W]]))

                        og = []
                        for b in range(NB):
                            accb = ps.tile([P, 2 * W], F32, tag="acc",
                                           bufs=2, name=f"acc{r}_{b}")
                            # head groups inside the block: a pair then
                            # (for 3-head blocks) a single
                            groups = ([(0, 1), (2,)] if HB_HEADS[b] == 3
                                      else [(0, 1)])
                            for us in groups:
                                for jt in range(2):
                                    gw = len(us) * W
                                    dots = ps.tile([P, 2 * W], F32,
                                                   tag="big", bufs=4,
                                                   name="dots")
                                    for i_u, u in enumerate(us):
                                        h = 3 * b + u
                                        ho = 32 * u
                                        cs = slice(i_u * W, (i_u + 1) * W)
                                        nc.tensor.matmul(
                                            dots[:, cs], ident[:],
                                            bt_ap(h, jt),
                                            start=(i_u == 0), stop=False,
                                            skip_group_check=True)
                                        nc.tensor.matmul(
                                            dots[:, cs],
                                            k_sb[b][ho:ho + DH,
                                                    rl * W + jt * P:
                                                    rl * W + (jt + 1) * P],
                                            q_sb[b][ho:ho + DH,
                                                    rl * W:(rl + 1) * W],
                                            start=False,
                                            stop=(i_u == len(us) - 1),
                                            skip_group_check=True)
                                    et = expp.tile([P, 2 * W], BF16,
                                                   tag="et")
                                    nc.scalar.activation(
                                        et[:, 0:gw], dots[:, 0:gw], AF.Exp,
                                        bias=ngj[:, 2 * r + jt:
                                                 2 * r + jt + 1])
                                    for i_u, u in enumerate(us):
                                        ho = 32 * u
                                        cs = slice(i_u * W, (i_u + 1) * W)
                                        nc.tensor.matmul(
                                            accb[ho:ho + DH, W:2 * W],
                                            ones_blk[:], et[:, cs],
                                            start=(jt == 0), stop=False,
                                            skip_group_check=True)
                                        nc.tensor.matmul(
                                            accb[ho:ho + DH, 0:W],
                                            v_sb[2 * rl + jt][
                                                :, b * P + ho:
                                                b * P + ho + DH],
                                            et[:, cs],
                                            start=False, stop=(jt == 1),
                                            skip_group_check=True)
                            # epilogue for block b
                            hbr = HB_ROWS[b]
                            rb = rowp.tile([P, W], F32, tag="rb")
                            nc.vector.reciprocal(rb[0:hbr],
                                                 accb[0:hbr, W:2 * W])
                            og0 = rowp.tile([P, W], BF16, tag="og0")
                            nc.vector.scalar_tensor_tensor(
                                out=og0[0:hbr], in0=accb[0:hbr, 0:W],
                                scalar=1.0, in1=rb[0:hbr],
                                op0=MUL, op1=MUL)
                            ogb = rowp.tile([P, W], BF16, tag=f"og{b}")
                            nc.gpsimd.tensor_tensor(
                                out=ogb[0:hbr], in0=og0[0:hbr],
                                in1=sig_sb[b][0:hbr, rl * W:(rl + 1) * W],
                                op=MUL)
                            vbs = rowp.tile([P, W], BF16, tag="vbs")
                            nc.vector.tensor_scalar(
                                out=vbs[0:hbr],
                                in0=sig_sb[b][0:hbr, rl * W:(rl + 1) * W],
                                scalar1=vbar_sb[0:hbr, b * 2 + rl:b * 2 + rl + 1],
                                scalar2=None,
                                op0=MUL)
                            nc.vector.copy_predicated(out=ogb[0:hbr],
                                                      mask=invm_b[0:hbr],
                                                      data=vbs[0:hbr])
                            og.append(ogb)

                        # ---- output projection for this row
                        ot = rowp.tile([P, 2, DN], F32, tag="ot")
                        for ts in range(2):
                            op = ps.tile([P, DN], F32, tag="big",
                                         bufs=4, name="op")
                            nc.tensor.matmul(op[:], ones_row[:], bo[:],
                                             start=True, stop=False)
                            for b in range(NB):
                                nc.tensor.matmul(
                                    op[:],
                                    og[b][0:HB_ROWS[b],
                                          ts * P:(ts + 1) * P],
                                    wo[b][:], start=False, stop=(b == NB - 1))
                            if ts == 0:
                                nc.vector.tensor_copy(out=ot[:, ts, :],
                                                      in_=op)
                            else:
                                nc.scalar.copy(out=ot[:, ts, :], in_=op)
                        nc.sync.dma_start(
                            out=bass.AP(tensor=o_d.tensor,
                                        offset=(c0 + rl * W) * DN,
                                        ap=[[DN, P], [P * DN, 2], [1, DN]]),
                            in_=ot)
    nc.compile()
    return nc


_NC_CACHE = {}
TRACE = False
REPEAT = 1
SIM_TRACE = False


def _get_nc(name):
    key = (name, REPEAT)
    if key not in _NC_CACHE:
        _NC_CACHE[key] = (_build_bias_nc if name == "bias"
                          else _build_attn_nc)()
    return _NC_CACHE[key]


def _prep(x, edges, mask, edge_mask, ln_g, ln_b, lne_g, lne_b,
          W_edge, Wq, Wkv, Wg, bg, Wo, bo):
    f32 = np.float32
    x = np.asarray(x, f32)
    edges = np.asarray(edges, f32)
    mask_b = np.asarray(mask).astype(bool)
    edge_mask_b = np.asarray(edge_mask).astype(bool)
    ln_g = np.asarray(ln_g, f32); ln_b = np.asarray(ln_b, f32)
    lne_g = np.asarray(lne_g, f32); lne_b = np.asarray(lne_b, f32)
    W_edge = np.asarray(W_edge, f32)
    Wq = np.asarray(Wq, f32); Wkv = np.asarray(Wkv, f32)
    Wg = np.asarray(Wg, f32); bg = np.asarray(bg, f32)
    Wo = np.asarray(Wo, f32); bo = np.asarray(bo, f32)

    idm = np.eye(128, dtype=f32)

    # ---------------- kernel 1: bias from edges
    nc1 = _get_nc("bias")
    we = (lne_g[:, None] * W_edge).astype(f32)
    e_flat = edges.reshape(W, W, DE)
    in_maps1 = []
    for c in range(NC):
        in_maps1.append({
            "e": np.ascontiguousarray(
                e_flat[c * IPC:(c + 1) * IPC].reshape(IPC * W, DE)),
            "we": _bf16(we),
            "idm": _bf16(idm),
        })
    res1 = bass_utils.run_bass_kernel_spmd(nc1, in_maps1,
                                           core_ids=list(range(NC)),
                                           trace=TRACE)
    if TRACE:
        print("bias kernel exec_time_ns:", res1.exec_time_ns)
    bias = np.concatenate(
        [res1.results[c]["o"].reshape(H, IPC, W) for c in range(NC)],
        axis=1)  # [H, i, j]
    bias = bias + (lne_b @ W_edge)[:, None, None]
    bias = np.where(edge_mask_b[0][None], bias, NEG).astype(f32)
    biasT = np.ascontiguousarray(bias.transpose(0, 2, 1))  # [H, j, i]
    bt = np.ascontiguousarray(
        biasT.reshape(H, 2, 128, W).transpose(2, 0, 1, 3))

    # ---------------- kernel 2: attention
    nc2 = _get_nc("attn")
    scale = DH ** -0.5
    Wk_, Wv_ = Wkv[:, :H * DH], Wkv[:, H * DH:]
    gq = _expand_cols((ln_g[:, None] * Wq * scale).astype(f32))
    gk = _expand_cols((ln_g[:, None] * Wk_).astype(f32))
    gv = _expand_cols((ln_g[:, None] * Wv_).astype(f32))
    gg = _expand_cols((ln_g[:, None] * Wg).astype(f32))
    assert np.allclose(ln_b, 0.0), "ln_b folding not implemented"
    # bg folded into the tanh trick: sigmoid(z+bg) = .5*tanh((z+bg)/2)+.5
    bgx = np.zeros((128, NB), f32)
    for h in range(H):
        bgx[32 * (h % 3):32 * (h % 3) + DH, h // 3] = \
            bg[h * DH:(h + 1) * DH] / 2.0
    woe = _expand_rows(Wo.astype(f32))

    maskf = mask_b[0].astype(f32)  # [R, W]
    x_flat = x.reshape(R, W, DN)
    in_maps2 = []
    for c in range(NC):
        mrows = maskf[c * RPC:(c + 1) * RPC]  # [RPC, W]
        ngj = (mrows.reshape(RPC, 2, 128) - 1.0) * 1e38  # [r, jt, p]
        ngj = np.ascontiguousarray(
            ngj.transpose(2, 0, 1).reshape(128, RPC * 2))
        in_maps2.append({
            "x": np.ascontiguousarray(
                x_flat[c * RPC:(c + 1) * RPC].reshape(RPC * W, DN)),
            "wq": _bf16(gq), "wk": _bf16(gk), "wv": _bf16(gv),
            "wg": _bf16(gg), "wo": _bf16(woe),
            "bg": bgx, "bo": _bf16(bo.reshape(1, DN)),
            "bt": _bf16(bt), "ngj": ngj.astype(f32),
            "idm": _bf16(idm),
            "invm": (1.0 - mrows).astype(np.uint8),
        })
    return nc2, in_maps2


def kernel(**inputs):
    nc2, in_maps2 = _prep(**inputs)
    res2 = bass_utils.run_bass_kernel_spmd(nc2, in_maps2,
                                           core_ids=list(range(NC)),
                                           trace=TRACE)
    if TRACE:
        print("attn kernel exec_time_ns:", res2.exec_time_ns)
    out = np.concatenate(
        [res2.results[c]["o"].reshape(RPC, W, DN) for c in range(NC)],
        axis=0)
    return out.reshape(B, R, W, DN).astype(np.float32)


# revision 14
# speedup vs baseline: 341.8433x; 341.8433x over previous
"""AxialAttention (MSA row attention) on 8 Trainium2 NeuronCores.

Sharding: data parallel over MSA rows r=128 (16 rows/core); the edge-bias
precompute is sharded over the edge i dim (32 i-rows/core) in kernel 1,
gathered on host, replicated into kernel 2 (attention).

v2 design notes (vs v1 baseline at 662us):
  - all matmuls in bf16 (v1 ran softmax-sum + attn@V matmuls as fp32 at
    4 cycles/row; bf16 is 1 cycle/row).
  - head layout: 2 blocks x 4 heads at partition offsets {0,32,64,96};
    SLOTS = 256 (v1: 3 blocks, 384).
  - denominator via ones-matmul replicated per head (32 rows), attn@V and
    denominator accumulate in one PSUM bank per block: av | den.
  - dots for a head PAIR share one PSUM tile [128, 512] -> one exp
    activation per pair (halves ACT instruction count).
  - ACT keeps ONE table resident (exp_and_others: Exp/Identity/Tanh):
    sigmoid(x) = 0.5*tanh(x/2)+0.5, LN rstd sqrt is batched into a single
    prologue op before the first exp (2 table loads total; v1 had 35).
  - LN stats in a prologue over SBUF-resident x; batched smalls.
  - few large DMAs instead of many small ones (HWDGE is 625ns/DMA).
"""

import sys
import numpy as np

sys.path.insert(0, "/opt/trn_rl_repo")

import concourse.bacc as bacc
import concourse.tile as tile
import concourse.bass as bass
from concourse import mybir
from concourse import bass_utils

F32 = mybir.dt.float32
BF16 = mybir.dt.bfloat16
U8 = mybir.dt.uint8
AF = mybir.ActivationFunctionType
MUL = mybir.AluOpType.mult
ADD = mybir.AluOpType.add

NC = 8
B, R, W, DN = 1, 128, 256, 256
DE, H, DH = 128, 8, 32
RPC = R // NC    # rows per core = 16
IPC = W // NC    # edge i-rows per core = 32
NEG = -1.0e38
EPS = 1e-5

NB = 3                  # head blocks (3/3/2 heads); base partition must be
SLOTS = NB * 128        # in {0,32,64}, so only 3 heads fit per 128-block
HB_ROWS = [96, 96, 64]  # used partitions per block
HB_HEADS = [3, 3, 2]


def _head_slot(h):
    return (h // 3) * 128 + 32 * (h % 3)


def _expand_cols(Wm):
    D = Wm.shape[0]
    out = np.zeros((D, SLOTS), Wm.dtype)
    for h in range(H):
        out[:, _head_slot(h):_head_slot(h) + DH] = Wm[:, h * DH:(h + 1) * DH]
    return out


def _expand_rows(Wm):
    D = Wm.shape[1]
    out = np.zeros((SLOTS, D), Wm.dtype)
    for h in range(H):
        out[_head_slot(h):_head_slot(h) + DH, :] = Wm[h * DH:(h + 1) * DH, :]
    return out


def _bf16(a):
    import ml_dtypes
    return np.asarray(a).astype(ml_dtypes.bfloat16)


# ---------------------------------------------------------------- kernel 1
def _build_bias_nc():
    """Per core: edges slice [IPC*W, DE] -> bias part [H, IPC*W]."""
    nc = bacc.Bacc("TRN2", target_bir_lowering=False, debug=False,
                   num_devices=NC)
    P = 128
    TOK = IPC * W          # 8192
    NT = TOK // P          # 64 token tiles
    e_d = nc.dram_tensor("e", [TOK, DE], F32, kind="ExternalInput").ap()
    we_d = nc.dram_tensor("we", [DE, H], BF16, kind="ExternalInput").ap()
    id_d = nc.dram_tensor("idm", [P, P], BF16, kind="ExternalInput").ap()
    o_d = nc.dram_tensor("o", [H, TOK], F32, kind="ExternalOutput").ap()

    with tile.TileContext(nc) as tc:
        with tc.tile_pool(name="cst", bufs=1) as cst, \
             tc.tile_pool(name="work", bufs=6) as work, \
             tc.tile_pool(name="tp", bufs=3) as tp, \
             tc.tile_pool(name="pst", bufs=3, space="PSUM") as pst, \
             tc.tile_pool(name="psb", bufs=2, space="PSUM") as psb:
            ident = cst.tile([P, P], BF16)
            nc.sync.dma_start(out=ident, in_=id_d)
            we_sb = cst.tile([DE, H], BF16)
            nc.sync.dma_start(out=we_sb, in_=we_d)
            eps_sb = cst.tile([P, 1], F32)
            nc.vector.memset(eps_sb, EPS)

            e_all = cst.tile([P, NT, DE], F32, name="e_all")
            for i in range(16):
                nc.sync.dma_start(
                    out=e_all[:, i * 4:(i + 1) * 4, :],
                    in_=bass.AP(tensor=e_d.tensor, offset=i * 4 * P * DE,
                                ap=[[DE, P], [P * DE, 4], [1, DE]]))

            mv = cst.tile([P, 2, NT], F32, name="mv")
            ost = cst.tile([H, TOK], F32, name="ost")

            from contextlib import nullcontext
            loop_ctx = (tc.For_i(0, REPEAT) if REPEAT > 1
                        else nullcontext(0))
            with loop_ctx:
                for t in range(NT):
                    st = work.tile([P, 6], F32, tag="st")
                    nc.vector.bn_stats(out=st, in_=e_all[:, t, :])
                    nc.vector.bn_aggr(out=mv[:, :, t], in_=st)
                # batched smalls
                sd = work.tile([P, NT], F32, tag="sd")
                nc.scalar.activation(sd, mv[:, 1, :], AF.Sqrt,
                                     bias=eps_sb[:])
                rstd = work.tile([P, NT], F32, tag="rs")
                nc.vector.reciprocal(rstd, sd)
                nmr = work.tile([P, NT], F32, tag="nm")
                nc.vector.scalar_tensor_tensor(
                    out=nmr, in0=mv[:, 0, :], scalar=-1.0, in1=rstd,
                    op0=MUL, op1=MUL)

                for g in range(NT // 4):
                    pt = pst.tile([DE, 4 * P], BF16, tag="pt")
                    for u in range(4):
                        t = g * 4 + u
                        en = work.tile([P, DE], BF16, tag="en")
                        nc.scalar.activation(en, e_all[:, t, :], AF.Identity,
                                             bias=nmr[:, t:t + 1],
                                             scale=rstd[:, t:t + 1])
                        nc.tensor.matmul(pt[:, u * P:(u + 1) * P],
                                         en[:], ident[:],
                                         is_transpose=True,
                                         start=(u == 0), stop=(u == 3))
                    enT = tp.tile([DE, 4 * P], BF16, tag="enT")
                    if g % 2 == 0:
                        nc.vector.tensor_copy(out=enT, in_=pt)
                    else:
                        nc.scalar.copy(out=enT, in_=pt)
                    ob = psb.tile([H, 4 * P], F32, tag="ob")
                    nc.tensor.matmul(ob[:], we_sb[:], enT[:],
                                     start=True, stop=True)
                    if g % 2 == 0:
                        nc.scalar.copy(out=ost[:, g * 4 * P:(g + 1) * 4 * P],
                                       in_=ob)
                    else:
                        nc.vector.tensor_copy(
                            out=ost[:, g * 4 * P:(g + 1) * 4 * P], in_=ob)
                for i in range(2):
                    nc.sync.dma_start(
                        out=o_d[:, i * (TOK // 2):(i + 1) * (TOK // 2)],
                        in_=ost[:, i * (TOK // 2):(i + 1) * (TOK // 2)])
    nc.compile()
    return nc


# ---------------------------------------------------------------- kernel 2
def _build_attn_nc():
    nc = bacc.Bacc("TRN2", target_bir_lowering=False, debug=False,
                   num_devices=NC)
    P = 128
    TOK = RPC * W          # 4096 tokens per core
    NT = TOK // P          # 32 token tiles
    CH = 512               # tokens per chunk (2 rows)
    NCH = TOK // CH        # 8 chunks

    x_d = nc.dram_tensor("x", [TOK, DN], F32, kind="ExternalInput").ap()
    wq_d = nc.dram_tensor("wq", [DN, SLOTS], BF16, kind="ExternalInput").ap()
    wk_d = nc.dram_tensor("wk", [DN, SLOTS], BF16, kind="ExternalInput").ap()
    wv_d = nc.dram_tensor("wv", [DN, SLOTS], BF16, kind="ExternalInput").ap()
    wg_d = nc.dram_tensor("wg", [DN, SLOTS], BF16, kind="ExternalInput").ap()
    wo_d = nc.dram_tensor("wo", [SLOTS, DN], BF16, kind="ExternalInput").ap()
    bg_d = nc.dram_tensor("bg", [P, NB], F32, kind="ExternalInput").ap()
    bo_d = nc.dram_tensor("bo", [1, DN], BF16, kind="ExternalInput").ap()
    bt_d = nc.dram_tensor("bt", [P, H, 2, W], BF16, kind="ExternalInput").ap()
    id_d = nc.dram_tensor("idm", [P, P], BF16, kind="ExternalInput").ap()
    ngj_d = nc.dram_tensor("ngj", [P, RPC * 2], F32,
                           kind="ExternalInput").ap()
    invm_d = nc.dram_tensor("invm", [RPC, W], U8, kind="ExternalInput").ap()
    o_d = nc.dram_tensor("o", [TOK, DN], F32, kind="ExternalOutput").ap()

    with tile.TileContext(nc, trace_sim=SIM_TRACE) as tc:
        from contextlib import ExitStack
        with ExitStack() as ctx:
            cst = ctx.enter_context(tc.tile_pool(name="cst", bufs=1))
            work = ctx.enter_context(tc.tile_pool(name="work", bufs=6))
            chw = ctx.enter_context(tc.tile_pool(name="chw", bufs=2))
            expp = ctx.enter_context(tc.tile_pool(name="expp", bufs=4))
            rowp = ctx.enter_context(tc.tile_pool(name="rowp", bufs=3))
            ps = ctx.enter_context(
                tc.tile_pool(name="ps", bufs=1, space="PSUM"))

            ident = cst.tile([P, P], BF16)
            nc.sync.dma_start(out=ident, in_=id_d)
            ones_blk = cst.tile([P, DH], BF16)
            nc.vector.memset(ones_blk, 1.0)
            ones_row = cst.tile([1, P], BF16)
            nc.vector.memset(ones_row, 1.0)
            wns = cst.tile([P, 1], BF16)
            nc.vector.memset(wns, 1.0 / W)
            eps_sb = cst.tile([P, 1], F32)
            nc.vector.memset(eps_sb, EPS)

            def load_w(d, shape, nm, dt=BF16):
                t = cst.tile(shape, dt, tag=nm, name=nm)
                nc.sync.dma_start(out=t, in_=d)
                return t

            wq = [load_w(wq_d[kt * P:(kt + 1) * P, :], [P, SLOTS], f"wq{kt}")
                  for kt in range(2)]
            wk = [load_w(wk_d[kt * P:(kt + 1) * P, :], [P, SLOTS], f"wk{kt}")
                  for kt in range(2)]
            wv = [load_w(wv_d[kt * P:(kt + 1) * P, :], [P, SLOTS], f"wv{kt}")
                  for kt in range(2)]
            wg = [load_w(wg_d[kt * P:(kt + 1) * P, :], [P, SLOTS], f"wg{kt}")
                  for kt in range(2)]
            wo = [load_w(wo_d[b * P:b * P + HB_ROWS[b], :],
                         [HB_ROWS[b], DN], f"wo{b}") for b in range(NB)]
            bg = load_w(bg_d, [P, NB], "bgt", F32)
            bo = load_w(bo_d, [1, DN], "bot")
            bt_sb = load_w(bt_d, [P, H * 2 * W], "btt")

            def bt_ap(h, jt):
                # [128, W] slice for head h, j-tile jt
                return bt_sb[:, (h * 2 + jt) * W:(h * 2 + jt + 1) * W]

            ngj = load_w(ngj_d, [P, RPC * 2], "ngjt", F32)

            x_all = cst.tile([P, NT, DN], F32, name="x_all")
            for i in range(8):
                nc.sync.dma_start(
                    out=x_all[:, i * 4:(i + 1) * 4, :],
                    in_=bass.AP(tensor=x_d.tensor, offset=i * 4 * P * DN,
                                ap=[[DN, P], [P * DN, 4], [1, DN]]))

            mv = cst.tile([P, 2, NT], F32, name="mv")
            xnT = [cst.tile([P, TOK], BF16, name=f"xnT{kt}")
                   for kt in range(2)]

            from contextlib import nullcontext
            loop_ctx = (tc.For_i(0, REPEAT) if REPEAT > 1
                        else nullcontext(0))
            with loop_ctx:
                # ---- P1: LN stats (batched smalls; single Sqrt before exp)
                for t in range(NT):
                    st = work.tile([P, 6], F32, tag="st")
                    nc.vector.bn_stats(out=st, in_=x_all[:, t, :])
                    nc.vector.bn_aggr(out=mv[:, :, t], in_=st)
                sd = work.tile([P, NT], F32, tag="sd")
                nc.scalar.activation(sd, mv[:, 1, :], AF.Sqrt, bias=eps_sb[:])
                rstd = work.tile([P, NT], F32, tag="rs")
                nc.vector.reciprocal(rstd, sd)
                nmr = work.tile([P, NT], F32, tag="nm")
                nc.vector.scalar_tensor_tensor(
                    out=nmr, in0=mv[:, 0, :], scalar=-1.0, in1=rstd,
                    op0=MUL, op1=MUL)

                # ---- P2: xn + transpose -> xnT [2][128, TOK] bf16
                for g in range(NT // 4):
                    pt = [ps.tile([P, 4 * P], BF16, tag="tr", bufs=1,
                                  name=f"pt{g}_{kt}")
                          for kt in range(2)]
                    for u in range(4):
                        t = g * 4 + u
                        xn = work.tile([P, DN], BF16, tag="xn")
                        nc.scalar.activation(xn, x_all[:, t, :], AF.Identity,
                                             bias=nmr[:, t:t + 1],
                                             scale=rstd[:, t:t + 1])
                        for kt in range(2):
                            nc.tensor.matmul(pt[kt][:, u * P:(u + 1) * P],
                                             xn[:, kt * P:(kt + 1) * P],
                                             ident[:], is_transpose=True,
                                             start=(u == 0), stop=(u == 3))
                    for kt in range(2):
                        dst = xnT[kt][:, g * 4 * P:(g + 1) * 4 * P]
                        if (g + kt) % 2 == 0:
                            nc.vector.tensor_copy(out=dst, in_=pt[kt])
                        else:
                            nc.scalar.copy(out=dst, in_=pt[kt])

                # ---- P3: chunk loop
                for ch in range(NCH):
                    c0 = ch * CH
                    q_sb, k_sb, sig_sb, v_sb = [], [], [], []
                    for b in range(NB):
                        pp = ps.tile([P, CH], F32, tag="big", bufs=4, name="pp")
                        for kt in range(2):
                            nc.tensor.matmul(
                                pp[:], wq[kt][:, b * P:(b + 1) * P],
                                xnT[kt][:, c0:c0 + CH],
                                start=(kt == 0), stop=(kt == 1))
                        t = chw.tile([P, CH], BF16, tag=f"q{b}")
                        nc.vector.tensor_copy(out=t, in_=pp)
                        q_sb.append(t)
                    for b in range(NB):
                        pp = ps.tile([P, CH], F32, tag="big", bufs=4, name="pp")
                        for kt in range(2):
                            nc.tensor.matmul(
                                pp[:], wk[kt][:, b * P:(b + 1) * P],
                                xnT[kt][:, c0:c0 + CH],
                                start=(kt == 0), stop=(kt == 1))
                        t = chw.tile([P, CH], BF16, tag=f"k{b}")
                        nc.scalar.copy(out=t, in_=pp)
                        k_sb.append(t)
                    for b in range(NB):
                        pp = ps.tile([P, CH], F32, tag="big", bufs=4, name="pp")
                        for kt in range(2):
                            nc.tensor.matmul(
                                pp[:], wg[kt][:, b * P:(b + 1) * P],
                                xnT[kt][:, c0:c0 + CH],
                                start=(kt == 0), stop=(kt == 1))
                        # sigmoid(z) = 0.5*tanh(z/2) + 0.5 (bg/2 folded in)
                        tnh = work.tile([P, CH], BF16, tag=f"tn{b}")
                        nc.scalar.activation(tnh, pp, AF.Tanh,
                                             bias=bg[:, b:b + 1], scale=0.5)
                        sig = chw.tile([P, CH], BF16, tag=f"s{b}")
                        nc.gpsimd.tensor_scalar(out=sig, in0=tnh,
                                                scalar1=0.5, scalar2=0.5,
                                                op0=MUL, op1=ADD)
                        sig_sb.append(sig)
                    for tb in range(4):
                        pp = ps.tile([P, SLOTS], F32, tag="big", bufs=4, name="pp")
                        for kt in range(2):
                            nc.tensor.matmul(
                                pp[:], xnT[kt][:, c0 + tb * P:
                                               c0 + (tb + 1) * P],
                                wv[kt][:], start=(kt == 0), stop=(kt == 1))
                        t = chw.tile([P, SLOTS], BF16, tag=f"v{tb}")
                        if tb % 2 == 0:
                            nc.vector.tensor_copy(out=t, in_=pp)
                        else:
                            nc.scalar.copy(out=t, in_=pp)
                        v_sb.append(t)

                    # vbar for both rows of the chunk: one PSUM group
                    vbp = ps.tile([P, NB * 2], F32, tag="vb", bufs=1,
                                  name=f"vb{ch}")
                    nmm = 0
                    for b in range(NB):
                        for rl2 in range(2):
                            for jt in range(2):
                                nc.tensor.matmul(
                                    vbp[:, b * 2 + rl2:b * 2 + rl2 + 1],
                                    v_sb[2 * rl2 + jt][:, b * P:(b + 1) * P],
                                    wns[:], start=(nmm == 0),
                                    stop=(nmm == NB * 4 - 1),
                                    skip_group_check=True)
                                nmm += 1
                    vbar_sb = chw.tile([P, NB * 2], F32, tag="vbar")
                    nc.vector.tensor_copy(out=vbar_sb, in_=vbp)

                    # ---- per-row attention
                    for rl in range(2):
                        r = ch * 2 + rl
                        invm_b = rowp.tile([P, W], U8, tag="invm")
                        nc.sync.dma_start(
                            out=invm_b,
                            in_=bass.AP(tensor=invm_d.tensor, offset=r * W,
                                        ap=[[0, P], [1, W]]))

                        og = []
                        for b in range(NB):
                            accb = ps.tile([P, 2 * W], F32, tag="acc",
                                           bufs=2, name=f"acc{r}_{b}")
                            # head groups inside the block: a pair then
                            # (for 3-head blocks) a single
                            groups = ([(0, 1), (2,)] if HB_HEADS[b] == 3
                                      else [(0, 1)])
                            for us in groups:
                                for jt in range(2):
                                    gw = len(us) * W
                                    dots = ps.tile([P, 2 * W], F32,
                                                   tag="big", bufs=4,
                                                   name="dots")
                                    for i_u, u in enumerate(us):
                                        h = 3 * b + u
                                        ho = 32 * u
                                        cs = slice(i_u * W, (i_u + 1) * W)
                                        nc.tensor.matmul(
                                            dots[:, cs], ident[:],
                                            bt_ap(h, jt),
                                            start=(i_u == 0), stop=False,
                                            skip_group_check=True)
                                        nc.tensor.matmul(
                                            dots[:, cs],
                                            k_sb[b][ho:ho + DH,
                                                    rl * W + jt * P:
                                                    rl * W + (jt + 1) * P],
                                            q_sb[b][ho:ho + DH,
                                                    rl * W:(rl + 1) * W],
                                            start=False,
                                            stop=(i_u == len(us) - 1),
                                            skip_group_check=True)
                                    et = expp.tile([P, 2 * W], BF16,
                                                   tag="et")
                                    nc.scalar.activation(
                                        et[:, 0:gw], dots[:, 0:gw], AF.Exp,
                                        bias=ngj[:, 2 * r + jt:
                                                 2 * r + jt + 1])
                                    for i_u, u in enumerate(us):
                                        ho = 32 * u
                                        cs = slice(i_u * W, (i_u + 1) * W)
                                        nc.tensor.matmul(
                                            accb[ho:ho + DH, W:2 * W],
                                            ones_blk[:], et[:, cs],
                                            start=(jt == 0), stop=False,
                                            skip_group_check=True)
                                        nc.tensor.matmul(
                                            accb[ho:ho + DH, 0:W],
                                            v_sb[2 * rl + jt][
                                                :, b * P + ho:
                                                b * P + ho + DH],
                                            et[:, cs],
                                            start=False, stop=(jt == 1),
                                            skip_group_check=True)
                            # epilogue for block b
                            hbr = HB_ROWS[b]
                            rb = rowp.tile([P, W], F32, tag="rb")
                            nc.vector.reciprocal(rb[0:hbr],
                                                 accb[0:hbr, W:2 * W])
                            og0 = rowp.tile([P, W], BF16, tag="og0")
                            nc.vector.scalar_tensor_tensor(
                                out=og0[0:hbr], in0=accb[0:hbr, 0:W],
                                scalar=1.0, in1=rb[0:hbr],
                                op0=MUL, op1=MUL)
                            ogb = rowp.tile([P, W], BF16, tag=f"og{b}")
                            nc.gpsimd.tensor_tensor(
                                out=ogb[0:hbr], in0=og0[0:hbr],
                                in1=sig_sb[b][0:hbr, rl * W:(rl + 1) * W],
                                op=MUL)
                            vbs = rowp.tile([P, W], BF16, tag="vbs")
                            nc.vector.tensor_scalar(
                                out=vbs[0:hbr],
                                in0=sig_sb[b][0:hbr, rl * W:(rl + 1) * W],
                                scalar1=vbar_sb[0:hbr, b * 2 + rl:b * 2 + rl + 1],
                                scalar2=None,
                                op0=MUL)
                            nc.vector.copy_predicated(out=ogb[0:hbr],
                                                      mask=invm_b[0:hbr],
                                                      data=vbs[0:hbr])
                            og.append(ogb)

                        # ---- output projection for this row
                        ot = rowp.tile([P, 2, DN], F32, tag="ot")
                        for ts in range(2):
                            op = ps.tile([P, DN], F32, tag="big",
                                         bufs=4, name="op")
                            nc.tensor.matmul(op[:], ones_row[:], bo[:],
                                             start=True, stop=False)
                            for b in range(NB):
                                nc.tensor.matmul(
                                    op[:],
                                    og[b][0:HB_ROWS[b],
                                          ts * P:(ts + 1) * P],
                                    wo[b][:], start=False, stop=(b == NB - 1))
                            if ts == 0:
                                nc.vector.tensor_copy(out=ot[:, ts, :],
                                                      in_=op)
                            else:
                                nc.scalar.copy(out=ot[:, ts, :], in_=op)
                        nc.sync.dma_start(
                            out=bass.AP(tensor=o_d.tensor,
                                        offset=(c0 + rl * W) * DN,
                                        ap=[[DN, P], [P * DN, 2], [1, DN]]),
                            in_=ot)
    nc.compile()
    return nc


_NC_CACHE = {}
TRACE = False
REPEAT = 1
SIM_TRACE = False


def _get_nc(name):
    key = (name, REPEAT)
    if key not in _NC_CACHE:
        _NC_CACHE[key] = (_build_bias_nc if name == "bias"
                          else _build_attn_nc)()
    return _NC_CACHE[key]


def _prep(x, edges, mask, edge_mask, ln_g, ln_b, lne_g, lne_b,
          W_edge, Wq, Wkv, Wg, bg, Wo, bo):
    f32 = np.float32
    x = np.asarray(x, f32)
    edges = np.asarray(edges, f32)
    mask_b = np.asarray(mask).astype(bool)
    edge_mask_b = np.asarray(edge_mask).astype(bool)
    ln_g = np.asarray(ln_g, f32); ln_b = np.asarray(ln_b, f32)
    lne_g = np.asarray(lne_g, f32); lne_b = np.asarray(lne_b, f32)
    W_edge = np.asarray(W_edge, f32)
    Wq = np.asarray(Wq, f32); Wkv = np.asarray(Wkv, f32)
    Wg = np.asarray(Wg, f32); bg = np.asarray(bg, f32)
    Wo = np.asarray(Wo, f32); bo = np.asarray(bo, f32)

    idm = np.eye(128, dtype=f32)

    # ---------------- kernel 1: bias from edges
    nc1 = _get_nc("bias")
    we = (lne_g[:, None] * W_edge).astype(f32)
    e_flat = edges.reshape(W, W, DE)
    in_maps1 = []
    for c in range(NC):
        in_maps1.append({
            "e": np.ascontiguousarray(
                e_flat[c * IPC:(c + 1) * IPC].reshape(IPC * W, DE)),
            "we": _bf16(we),
            "idm": _bf16(idm),
        })
    res1 = bass_utils.run_bass_kernel_spmd(nc1, in_maps1,
                                           core_ids=list(range(NC)),
                                           trace=TRACE)
    if TRACE:
        print("bias kernel exec_time_ns:", res1.exec_time_ns)
    bias = np.concatenate(
        [res1.results[c]["o"].reshape(H, IPC, W) for c in range(NC)],
        axis=1)  # [H, i, j]
    bias = bias + (lne_b @ W_edge)[:, None, None]
    bias = np.where(edge_mask_b[0][None], bias, NEG).astype(f32)
    biasT = np.ascontiguousarray(bias.transpose(0, 2, 1))  # [H, j, i]
    bt = np.ascontiguousarray(
        biasT.reshape(H, 2, 128, W).transpose(2, 0, 1, 3))

    # ---------------- kernel 2: attention
    nc2 = _get_nc("attn")
    scale = DH ** -0.5
    Wk_, Wv_ = Wkv[:, :H * DH], Wkv[:, H * DH:]
    gq = _expand_cols((ln_g[:, None] * Wq * scale).astype(f32))
    gk = _expand_cols((ln_g[:, None] * Wk_).astype(f32))
    gv = _expand_cols((ln_g[:, None] * Wv_).astype(f32))
    gg = _expand_cols((ln_g[:, None] * Wg).astype(f32))
    assert np.allclose(ln_b, 0.0), "ln_b folding not implemented"
    # bg folded into the tanh trick: sigmoid(z+bg) = .5*tanh((z+bg)/2)+.5
    bgx = np.zeros((128, NB), f32)
    for h in range(H):
        bgx[32 * (h % 3):32 * (h % 3) + DH, h // 3] = \
            bg[h * DH:(h + 1) * DH] / 2.0
    woe = _expand_rows(Wo.astype(f32))

    maskf = mask_b[0].astype(f32)  # [R, W]
    x_flat = x.reshape(R, W, DN)
    in_maps2 = []
    for c in range(NC):
        mrows = maskf[c * RPC:(c + 1) * RPC]  # [RPC, W]
        ngj = (mrows.reshape(RPC, 2, 128) - 1.0) * 1e38  # [r, jt, p]
        ngj = np.ascontiguousarray(
            ngj.transpose(2, 0, 1).reshape(128, RPC * 2))
        in_maps2.append({
            "x": np.ascontiguousarray(
                x_flat[c * RPC:(c + 1) * RPC].reshape(RPC * W, DN)),
            "wq": _bf16(gq), "wk": _bf16(gk), "wv": _bf16(gv),
            "wg": _bf16(gg), "wo": _bf16(woe),
            "bg": bgx, "bo": _bf16(bo.reshape(1, DN)),
            "bt": _bf16(bt), "ngj": ngj.astype(f32),
            "idm": _bf16(idm),
            "invm": (1.0 - mrows).astype(np.uint8),
        })
    return nc2, in_maps2


def kernel(**inputs):
    nc2, in_maps2 = _prep(**inputs)
    res2 = bass_utils.run_bass_kernel_spmd(nc2, in_maps2,
                                           core_ids=list(range(NC)),
                                           trace=TRACE)
    if TRACE:
        print("attn kernel exec_time_ns:", res2.exec_time_ns)
    out = np.concatenate(
        [res2.results[c]["o"].reshape(RPC, W, DN) for c in range(NC)],
        axis=0)
    return out.reshape(B, R, W, DN).astype(np.float32)


# revision 18
# speedup vs baseline: 472.0186x; 1.3808x over previous
"""AxialAttention (MSA row attention) on 8 Trainium2 NeuronCores.

Sharding: data parallel over MSA rows r=128 (16 rows/core); the edge-bias
precompute is sharded over the edge i dim (32 i-rows/core) in kernel 1,
gathered on host, replicated into kernel 2 (attention).

v2 design notes (vs v1 baseline at 662us):
  - all matmuls in bf16 (v1 ran softmax-sum + attn@V matmuls as fp32 at
    4 cycles/row; bf16 is 1 cycle/row).
  - head layout: 2 blocks x 4 heads at partition offsets {0,32,64,96};
    SLOTS = 256 (v1: 3 blocks, 384).
  - denominator via ones-matmul replicated per head (32 rows), attn@V and
    denominator accumulate in one PSUM bank per block: av | den.
  - dots for a head PAIR share one PSUM tile [128, 512] -> one exp
    activation per pair (halves ACT instruction count).
  - ACT keeps ONE table resident (exp_and_others: Exp/Identity/Tanh):
    sigmoid(x) = 0.5*tanh(x/2)+0.5, LN rstd sqrt is batched into a single
    prologue op before the first exp (2 table loads total; v1 had 35).
  - LN stats in a prologue over SBUF-resident x; batched smalls.
  - few large DMAs instead of many small ones (HWDGE is 625ns/DMA).
"""

import sys
import numpy as np

sys.path.insert(0, "/opt/trn_rl_repo")

import concourse.bacc as bacc
import concourse.tile as tile
import concourse.bass as bass
from concourse import mybir
from concourse import bass_utils

F32 = mybir.dt.float32
BF16 = mybir.dt.bfloat16
U8 = mybir.dt.uint8
AF = mybir.ActivationFunctionType
MUL = mybir.AluOpType.mult
ADD = mybir.AluOpType.add

NC = 8
B, R, W, DN = 1, 128, 256, 256
DE, H, DH = 128, 8, 32
RPC = R // NC    # rows per core = 16
IPC = W // NC    # edge i-rows per core = 32
NEG = -1.0e38
EPS = 1e-5

NB = 3                  # head blocks (3/3/2 heads); base partition must be
SLOTS = NB * 128        # in {0,32,64}, so only 3 heads fit per 128-block
HB_ROWS = [96, 96, 64]  # used partitions per block
HB_HEADS = [3, 3, 2]


def _head_slot(h):
    return (h // 3) * 128 + 32 * (h % 3)


def _expand_cols(Wm):
    D = Wm.shape[0]
    out = np.zeros((D, SLOTS), Wm.dtype)
    for h in range(H):
        out[:, _head_slot(h):_head_slot(h) + DH] = Wm[:, h * DH:(h + 1) * DH]
    return out


def _expand_rows(Wm):
    D = Wm.shape[1]
    out = np.zeros((SLOTS, D), Wm.dtype)
    for h in range(H):
        out[_head_slot(h):_head_slot(h) + DH, :] = Wm[h * DH:(h + 1) * DH, :]
    return out


def _bf16(a):
    import ml_dtypes
    return np.asarray(a).astype(ml_dtypes.bfloat16)


# ---------------------------------------------------------------- kernel 1
def _build_bias_nc():
    """Per core: edges slice [IPC*W, DE] -> bias part [H, IPC*W]."""
    nc = bacc.Bacc("TRN2", target_bir_lowering=False, debug=False,
                   num_devices=NC)
    P = 128
    TOK = IPC * W          # 8192
    NT = TOK // P          # 64 token tiles
    e_d = nc.dram_tensor("e", [TOK, DE], F32, kind="ExternalInput").ap()
    we_d = nc.dram_tensor("we", [DE, H], BF16, kind="ExternalInput").ap()
    id_d = nc.dram_tensor("idm", [P, P], BF16, kind="ExternalInput").ap()
    o_d = nc.dram_tensor("o", [H, TOK], F32, kind="ExternalOutput").ap()

    with tile.TileContext(nc) as tc:
        with tc.tile_pool(name="cst", bufs=1) as cst, \
             tc.tile_pool(name="work", bufs=6) as work, \
             tc.tile_pool(name="tp", bufs=3) as tp, \
             tc.tile_pool(name="pst", bufs=3, space="PSUM") as pst, \
             tc.tile_pool(name="psb", bufs=2, space="PSUM") as psb:
            ident = cst.tile([P, P], BF16)
            nc.sync.dma_start(out=ident, in_=id_d)
            we_sb = cst.tile([DE, H], BF16)
            nc.sync.dma_start(out=we_sb, in_=we_d)
            eps_sb = cst.tile([P, 1], F32)
            nc.vector.memset(eps_sb, EPS)

            e_all = cst.tile([P, NT, DE], F32, name="e_all")
            for i in range(16):
                nc.sync.dma_start(
                    out=e_all[:, i * 4:(i + 1) * 4, :],
                    in_=bass.AP(tensor=e_d.tensor, offset=i * 4 * P * DE,
                                ap=[[DE, P], [P * DE, 4], [1, DE]]))

            mv = cst.tile([P, 2, NT], F32, name="mv")
            ost = cst.tile([H, TOK], F32, name="ost")

            from contextlib import nullcontext
            loop_ctx = (tc.For_i(0, REPEAT) if REPEAT > 1
                        else nullcontext(0))
            with loop_ctx:
                for t in range(NT):
                    st = work.tile([P, 6], F32, tag="st")
                    nc.vector.bn_stats(out=st, in_=e_all[:, t, :])
                    nc.vector.bn_aggr(out=mv[:, :, t], in_=st)
                # batched smalls
                sd = work.tile([P, NT], F32, tag="sd")
                nc.scalar.activation(sd, mv[:, 1, :], AF.Sqrt,
                                     bias=eps_sb[:])
                rstd = work.tile([P, NT], F32, tag="rs")
                nc.vector.reciprocal(rstd, sd)
                nmr = work.tile([P, NT], F32, tag="nm")
                nc.vector.scalar_tensor_tensor(
                    out=nmr, in0=mv[:, 0, :], scalar=-1.0, in1=rstd,
                    op0=MUL, op1=MUL)

                for g in range(NT // 4):
                    pt = pst.tile([DE, 4 * P], BF16, tag="pt")
                    for u in range(4):
                        t = g * 4 + u
                        en = work.tile([P, DE], BF16, tag="en")
                        nc.scalar.activation(en, e_all[:, t, :], AF.Identity,
                                             bias=nmr[:, t:t + 1],
                                             scale=rstd[:, t:t + 1])
                        nc.tensor.matmul(pt[:, u * P:(u + 1) * P],
                                         en[:], ident[:],
                                         is_transpose=True,
                                         start=(u == 0), stop=(u == 3))
                    enT = tp.tile([DE, 4 * P], BF16, tag="enT")
                    if g % 2 == 0:
                        nc.vector.tensor_copy(out=enT, in_=pt)
                    else:
                        nc.scalar.copy(out=enT, in_=pt)
                    ob = psb.tile([H, 4 * P], F32, tag="ob")
                    nc.tensor.matmul(ob[:], we_sb[:], enT[:],
                                     start=True, stop=True)
                    if g % 2 == 0:
                        nc.scalar.copy(out=ost[:, g * 4 * P:(g + 1) * 4 * P],
                                       in_=ob)
                    else:
                        nc.vector.tensor_copy(
                            out=ost[:, g * 4 * P:(g + 1) * 4 * P], in_=ob)
                for i in range(2):
                    nc.sync.dma_start(
                        out=o_d[:, i * (TOK // 2):(i + 1) * (TOK // 2)],
                        in_=ost[:, i * (TOK // 2):(i + 1) * (TOK // 2)])
    nc.compile()
    return nc


# ---------------------------------------------------------------- kernel 2
def _build_attn_nc():
    nc = bacc.Bacc("TRN2", target_bir_lowering=False, debug=False,
                   num_devices=NC)
    P = 128
    TOK = RPC * W          # 4096 tokens per core
    NT = TOK // P          # 32 token tiles
    CH = 512               # tokens per chunk (2 rows)
    NCH = TOK // CH        # 8 chunks

    x_d = nc.dram_tensor("x", [TOK, DN], F32, kind="ExternalInput").ap()
    wq_d = nc.dram_tensor("wq", [DN, SLOTS], BF16, kind="ExternalInput").ap()
    wk_d = nc.dram_tensor("wk", [DN, SLOTS], BF16, kind="ExternalInput").ap()
    wv_d = nc.dram_tensor("wv", [DN, SLOTS], BF16, kind="ExternalInput").ap()
    wg_d = nc.dram_tensor("wg", [DN, SLOTS], BF16, kind="ExternalInput").ap()
    wo_d = nc.dram_tensor("wo", [SLOTS, DN], BF16, kind="ExternalInput").ap()
    bg_d = nc.dram_tensor("bg", [P, NB], F32, kind="ExternalInput").ap()
    bo_d = nc.dram_tensor("bo", [1, DN], BF16, kind="ExternalInput").ap()
    bt_d = nc.dram_tensor("bt", [P, H, 2, W], BF16, kind="ExternalInput").ap()
    id_d = nc.dram_tensor("idm", [P, P], BF16, kind="ExternalInput").ap()
    ngj_d = nc.dram_tensor("ngj", [P, RPC * 2], F32,
                           kind="ExternalInput").ap()
    invm_d = nc.dram_tensor("invm", [RPC, W], U8, kind="ExternalInput").ap()
    o_d = nc.dram_tensor("o", [TOK, DN], F32, kind="ExternalOutput").ap()

    with tile.TileContext(nc, trace_sim=SIM_TRACE) as tc:
        from contextlib import ExitStack
        with ExitStack() as ctx:
            cst = ctx.enter_context(tc.tile_pool(name="cst", bufs=1))
            work = ctx.enter_context(tc.tile_pool(name="work", bufs=6))
            chw = ctx.enter_context(tc.tile_pool(name="chw", bufs=2))
            expp = ctx.enter_context(tc.tile_pool(name="expp", bufs=4))
            rowp = ctx.enter_context(tc.tile_pool(name="rowp", bufs=3))
            ps = ctx.enter_context(
                tc.tile_pool(name="ps", bufs=1, space="PSUM"))

            ident = cst.tile([P, P], BF16)
            nc.sync.dma_start(out=ident, in_=id_d)
            ones_blk = cst.tile([P, DH], BF16)
            nc.vector.memset(ones_blk, 1.0)
            ones_row = cst.tile([1, P], BF16)
            nc.vector.memset(ones_row, 1.0)
            wns = cst.tile([P, 1], BF16)
            nc.vector.memset(wns, 1.0 / W)
            eps_sb = cst.tile([P, 1], F32)
            nc.vector.memset(eps_sb, EPS)

            def load_w(d, shape, nm, dt=BF16):
                t = cst.tile(shape, dt, tag=nm, name=nm)
                nc.sync.dma_start(out=t, in_=d)
                return t

            wq = [load_w(wq_d[kt * P:(kt + 1) * P, :], [P, SLOTS], f"wq{kt}")
                  for kt in range(2)]
            wk = [load_w(wk_d[kt * P:(kt + 1) * P, :], [P, SLOTS], f"wk{kt}")
                  for kt in range(2)]
            wv = [load_w(wv_d[kt * P:(kt + 1) * P, :], [P, SLOTS], f"wv{kt}")
                  for kt in range(2)]
            wg = [load_w(wg_d[kt * P:(kt + 1) * P, :], [P, SLOTS], f"wg{kt}")
                  for kt in range(2)]
            wo = [load_w(wo_d[b * P:b * P + HB_ROWS[b], :],
                         [HB_ROWS[b], DN], f"wo{b}") for b in range(NB)]
            bg = load_w(bg_d, [P, NB], "bgt", F32)
            bo = load_w(bo_d, [1, DN], "bot")
            bt_sb = load_w(bt_d, [P, H * 2 * W], "btt")

            def bt_ap(h, jt):
                # [128, W] slice for head h, j-tile jt
                return bt_sb[:, (h * 2 + jt) * W:(h * 2 + jt + 1) * W]

            ngj = load_w(ngj_d, [P, RPC * 2], "ngjt", F32)

            x_all = cst.tile([P, NT, DN], F32, name="x_all")
            for i in range(8):
                nc.sync.dma_start(
                    out=x_all[:, i * 4:(i + 1) * 4, :],
                    in_=bass.AP(tensor=x_d.tensor, offset=i * 4 * P * DN,
                                ap=[[DN, P], [P * DN, 4], [1, DN]]))

            mv = cst.tile([P, 2, NT], F32, name="mv")
            xnT = [cst.tile([P, TOK], BF16, name=f"xnT{kt}")
                   for kt in range(2)]

            from contextlib import nullcontext
            loop_ctx = (tc.For_i(0, REPEAT) if REPEAT > 1
                        else nullcontext(0))
            with loop_ctx:
                if BISECT == "empty":
                    dummy = work.tile([P, 8], F32, tag="dummy")
                    nc.vector.memset(dummy, 0.0)
                # ---- P1: LN stats (batched smalls; single Sqrt before exp)
                for t in range(NT if BISECT != "empty" else 0):
                    st = work.tile([P, 6], F32, tag="st")
                    nc.vector.bn_stats(out=st, in_=x_all[:, t, :])
                    nc.vector.bn_aggr(out=mv[:, :, t], in_=st)
                if BISECT == "empty":
                    sd = None
                else:
                    sd = work.tile([P, NT], F32, tag="sd")
                    nc.scalar.activation(sd, mv[:, 1, :], AF.Sqrt,
                                         bias=eps_sb[:])
                if BISECT != "empty":
                    rstd = work.tile([P, NT], F32, tag="rs")
                    nc.vector.reciprocal(rstd, sd)
                    nmr = work.tile([P, NT], F32, tag="nm")
                    nc.vector.scalar_tensor_tensor(
                        out=nmr, in0=mv[:, 0, :], scalar=-1.0, in1=rstd,
                        op0=MUL, op1=MUL)

                # ---- P2: xn + transpose -> xnT [2][128, TOK] bf16
                for g in range(NT // 4 if BISECT != "empty" else 0):
                    pt = [ps.tile([P, 4 * P], BF16, tag="tr", bufs=1,
                                  name=f"pt{g}_{kt}")
                          for kt in range(2)]
                    for u in range(4):
                        t = g * 4 + u
                        xn = work.tile([P, DN], BF16, tag="xn")
                        nc.scalar.activation(xn, x_all[:, t, :], AF.Identity,
                                             bias=nmr[:, t:t + 1],
                                             scale=rstd[:, t:t + 1])
                        for kt in range(2):
                            nc.tensor.matmul(pt[kt][:, u * P:(u + 1) * P],
                                             xn[:, kt * P:(kt + 1) * P],
                                             ident[:], is_transpose=True,
                                             start=(u == 0), stop=(u == 3))
                    for kt in range(2):
                        dst = xnT[kt][:, g * 4 * P:(g + 1) * 4 * P]
                        if (g + kt) % 2 == 0:
                            nc.vector.tensor_copy(out=dst, in_=pt[kt])
                        else:
                            nc.scalar.copy(out=dst, in_=pt[kt])

                # ---- P3: chunk loop
                for ch in range(NCH):
                    if BISECT in ("noproj", "empty"):
                        continue
                    c0 = ch * CH
                    q_sb, k_sb, tnh_sb, v_sb = [], [], [], []
                    for b in range(NB):
                        pp = ps.tile([P, CH], F32, tag="big", bufs=4, name="pp")
                        for kt in range(2):
                            nc.tensor.matmul(
                                pp[:], wq[kt][:, b * P:(b + 1) * P],
                                xnT[kt][:, c0:c0 + CH],
                                start=(kt == 0), stop=(kt == 1))
                        t = chw.tile([P, CH], BF16, tag=f"q{b}")
                        nc.vector.tensor_copy(out=t, in_=pp)
                        q_sb.append(t)
                    for b in range(NB):
                        pp = ps.tile([P, CH], F32, tag="big", bufs=4, name="pp")
                        for kt in range(2):
                            nc.tensor.matmul(
                                pp[:], wk[kt][:, b * P:(b + 1) * P],
                                xnT[kt][:, c0:c0 + CH],
                                start=(kt == 0), stop=(kt == 1))
                        t = chw.tile([P, CH], BF16, tag=f"k{b}")
                        nc.scalar.copy(out=t, in_=pp)
                        k_sb.append(t)
                    for b in range(NB):
                        pp = ps.tile([P, CH], F32, tag="big", bufs=4, name="pp")
                        for kt in range(2):
                            nc.tensor.matmul(
                                pp[:], wg[kt][:, b * P:(b + 1) * P],
                                xnT[kt][:, c0:c0 + CH],
                                start=(kt == 0), stop=(kt == 1))
                        # sigmoid(z) = 0.5*(tanh(z/2)+1); the 0.5 is
                        # folded into wo on the host, bg/2 into the bias
                        tnh = chw.tile([P, CH], BF16, tag=f"tn{b}")
                        nc.scalar.activation(tnh, pp, AF.Tanh,
                                             bias=bg[:, b:b + 1], scale=0.5)
                        tnh_sb.append(tnh)
                    for tb in range(4):
                        pp = ps.tile([P, SLOTS], F32, tag="big", bufs=4, name="pp")
                        for kt in range(2):
                            nc.tensor.matmul(
                                pp[:], xnT[kt][:, c0 + tb * P:
                                               c0 + (tb + 1) * P],
                                wv[kt][:], start=(kt == 0), stop=(kt == 1))
                        t = chw.tile([P, SLOTS], BF16, tag=f"v{tb}")
                        if tb % 2 == 0:
                            nc.vector.tensor_copy(out=t, in_=pp)
                        else:
                            nc.scalar.copy(out=t, in_=pp)
                        v_sb.append(t)

                    # vbar for both rows of the chunk: one PSUM group
                    vbp = ps.tile([P, NB * 2], F32, tag="vb", bufs=1,
                                  name=f"vb{ch}")
                    nmm = 0
                    for b in range(NB):
                        for rl2 in range(2):
                            for jt in range(2):
                                nc.tensor.matmul(
                                    vbp[:, b * 2 + rl2:b * 2 + rl2 + 1],
                                    v_sb[2 * rl2 + jt][:, b * P:(b + 1) * P],
                                    wns[:], start=(nmm == 0),
                                    stop=(nmm == NB * 4 - 1),
                                    skip_group_check=True)
                                nmm += 1
                    vbar_sb = chw.tile([P, NB * 2], F32, tag="vbar")
                    nc.vector.tensor_copy(out=vbar_sb, in_=vbp)

                    if BISECT == "norows":
                        continue
                    # ---- software-pipelined row attention (both rows).
                    # unit = (rl, b, us, jt); PE emits dots(n+1) before
                    # av/den(n) so ACT's exp(n) hides under PE work.
                    invms = []
                    for rl in range(2):
                        r = ch * 2 + rl
                        ib = rowp.tile([P, W], U8, tag=f"invm{rl}",
                                       name=f"invm{r}")
                        nc.sync.dma_start(
                            out=ib,
                            in_=bass.AP(tensor=invm_d.tensor, offset=r * W,
                                        ap=[[0, P], [1, W]]))
                        invms.append(ib)

                    units = []
                    for rl in range(2):
                        for b in range(NB):
                            groups = ([(0, 1), (2,)] if HB_HEADS[b] == 3
                                      else [(0, 1)])
                            for gi, us in enumerate(groups):
                                for jt in range(2):
                                    last = (gi == len(groups) - 1
                                            and jt == 1)
                                    units.append((rl, b, us, jt, last))

                    accs = {}
                    ogs = {0: [], 1: []}

                    def emit_dots_exp(unit):
                        rl, b, us, jt, _ = unit
                        r = ch * 2 + rl
                        gw = len(us) * W
                        dots = ps.tile([P, 2 * W], F32, tag="big", bufs=4,
                                       name="dots")
                        for i_u, u in enumerate(us):
                            h = 3 * b + u
                            ho = 32 * u
                            cs = slice(i_u * W, (i_u + 1) * W)
                            nc.tensor.matmul(
                                dots[:, cs], ident[:], bt_ap(h, jt),
                                start=(i_u == 0), stop=False,
                                skip_group_check=True)
                            nc.tensor.matmul(
                                dots[:, cs],
                                k_sb[b][ho:ho + DH,
                                        rl * W + jt * P:
                                        rl * W + (jt + 1) * P],
                                q_sb[b][ho:ho + DH, rl * W:(rl + 1) * W],
                                start=False, stop=(i_u == len(us) - 1),
                                skip_group_check=True)
                        et = expp.tile([P, 2 * W], BF16, tag="et")
                        nc.scalar.activation(
                            et[:, 0:gw], dots[:, 0:gw], AF.Exp,
                            bias=ngj[:, 2 * r + jt:2 * r + jt + 1])
                        return et

                    def emit_epilogue(rl, b, accb):
                        hbr = HB_ROWS[b]
                        rb = rowp.tile([P, W], F32, tag="rb")
                        nc.vector.reciprocal(rb[0:hbr],
                                             accb[0:hbr, W:2 * W])
                        og0 = rowp.tile([P, W], BF16, tag="og0")
                        nc.vector.scalar_tensor_tensor(
                            out=og0[0:hbr], in0=accb[0:hbr, 0:W],
                            scalar=1.0, in1=rb[0:hbr], op0=MUL, op1=MUL)
                        # og = og0 * (tanh + 1); the 0.5 lives in wo
                        ogb = rowp.tile([P, W], BF16, tag=f"og{b}")
                        nc.vector.scalar_tensor_tensor(
                            out=ogb[0:hbr],
                            in0=tnh_sb[b][0:hbr, rl * W:(rl + 1) * W],
                            scalar=1.0, in1=og0[0:hbr], op0=ADD, op1=MUL)
                        vbs = rowp.tile([P, W], BF16, tag="vbs")
                        nc.vector.tensor_scalar(
                            out=vbs[0:hbr],
                            in0=tnh_sb[b][0:hbr, rl * W:(rl + 1) * W],
                            scalar1=1.0,
                            scalar2=vbar_sb[0:hbr,
                                            b * 2 + rl:b * 2 + rl + 1],
                            op0=ADD, op1=MUL)
                        nc.vector.copy_predicated(out=ogb[0:hbr],
                                                  mask=invms[rl][0:hbr],
                                                  data=vbs[0:hbr])
                        ogs[rl].append(ogb)
                        if b == NB - 1:
                            emit_outproj(rl)

                    def emit_outproj(rl):
                        ot = rowp.tile([P, 2, DN], F32, tag=f"ot{rl}",
                                       name=f"ot{ch}_{rl}")
                        for ts in range(2):
                            op = ps.tile([P, DN], F32, tag="big", bufs=4,
                                         name="op")
                            nc.tensor.matmul(op[:], ones_row[:], bo[:],
                                             start=True, stop=False)
                            for b in range(NB):
                                nc.tensor.matmul(
                                    op[:],
                                    ogs[rl][b][0:HB_ROWS[b],
                                               ts * P:(ts + 1) * P],
                                    wo[b][:], start=False,
                                    stop=(b == NB - 1))
                            if ts == 0:
                                nc.vector.tensor_copy(out=ot[:, ts, :],
                                                      in_=op)
                            else:
                                nc.scalar.copy(out=ot[:, ts, :], in_=op)
                        nc.sync.dma_start(
                            out=bass.AP(tensor=o_d.tensor,
                                        offset=(c0 + rl * W) * DN,
                                        ap=[[DN, P], [P * DN, 2], [1, DN]]),
                            in_=ot)

                    def emit_avden(unit, et):
                        rl, b, us, jt, last = unit
                        if (rl, b) not in accs:
                            accs[(rl, b)] = ps.tile(
                                [P, 2 * W], F32, tag="acc", bufs=2,
                                name=f"acc{ch}_{rl}_{b}")
                        accb = accs[(rl, b)]
                        for i_u, u in enumerate(us):
                            ho = 32 * u
                            cs = slice(i_u * W, (i_u + 1) * W)
                            nc.tensor.matmul(
                                accb[ho:ho + DH, W:2 * W],
                                ones_blk[:], et[:, cs],
                                start=(jt == 0), stop=False,
                                skip_group_check=True)
                            nc.tensor.matmul(
                                accb[ho:ho + DH, 0:W],
                                v_sb[2 * rl + jt][:, b * P + ho:
                                                  b * P + ho + DH],
                                et[:, cs],
                                start=False, stop=(jt == 1),
                                skip_group_check=True)
                        if last:
                            emit_epilogue(rl, b, accb)

                    pend = None
                    for unit in units:
                        et = emit_dots_exp(unit)
                        if pend is not None:
                            emit_avden(*pend)
                        pend = (unit, et)
                    emit_avden(*pend)
    nc.compile()
    return nc


_NC_CACHE = {}
TRACE = False
REPEAT = 1
SIM_TRACE = False
import os
BISECT = os.environ.get("K_BISECT", "full")


def _get_nc(name):
    key = (name, REPEAT, BISECT)
    if key not in _NC_CACHE:
        _NC_CACHE[key] = (_build_bias_nc if name == "bias"
                          else _build_attn_nc)()
    return _NC_CACHE[key]


def _prep(x, edges, mask, edge_mask, ln_g, ln_b, lne_g, lne_b,
          W_edge, Wq, Wkv, Wg, bg, Wo, bo):
    f32 = np.float32
    x = np.asarray(x, f32)
    edges = np.asarray(edges, f32)
    mask_b = np.asarray(mask).astype(bool)
    edge_mask_b = np.asarray(edge_mask).astype(bool)
    ln_g = np.asarray(ln_g, f32); ln_b = np.asarray(ln_b, f32)
    lne_g = np.asarray(lne_g, f32); lne_b = np.asarray(lne_b, f32)
    W_edge = np.asarray(W_edge, f32)
    Wq = np.asarray(Wq, f32); Wkv = np.asarray(Wkv, f32)
    Wg = np.asarray(Wg, f32); bg = np.asarray(bg, f32)
    Wo = np.asarray(Wo, f32); bo = np.asarray(bo, f32)

    idm = np.eye(128, dtype=f32)

    # ---------------- kernel 1: bias from edges
    nc1 = _get_nc("bias")
    we = (lne_g[:, None] * W_edge).astype(f32)
    e_flat = edges.reshape(W, W, DE)
    in_maps1 = []
    for c in range(NC):
        in_maps1.append({
            "e": np.ascontiguousarray(
                e_flat[c * IPC:(c + 1) * IPC].reshape(IPC * W, DE)),
            "we": _bf16(we),
            "idm": _bf16(idm),
        })
    res1 = bass_utils.run_bass_kernel_spmd(nc1, in_maps1,
                                           core_ids=list(range(NC)),
                                           trace=TRACE)
    if TRACE:
        print("bias kernel exec_time_ns:", res1.exec_time_ns)
    bias = np.concatenate(
        [res1.results[c]["o"].reshape(H, IPC, W) for c in range(NC)],
        axis=1)  # [H, i, j]
    bias = bias + (lne_b @ W_edge)[:, None, None]
    bias = np.where(edge_mask_b[0][None], bias, NEG).astype(f32)
    biasT = np.ascontiguousarray(bias.transpose(0, 2, 1))  # [H, j, i]
    bt = np.ascontiguousarray(
        biasT.reshape(H, 2, 128, W).transpose(2, 0, 1, 3))

    # ---------------- kernel 2: attention
    nc2 = _get_nc("attn")
    scale = DH ** -0.5
    Wk_, Wv_ = Wkv[:, :H * DH], Wkv[:, H * DH:]
    gq = _expand_cols((ln_g[:, None] * Wq * scale).astype(f32))
    gk = _expand_cols((ln_g[:, None] * Wk_).astype(f32))
    gv = _expand_cols((ln_g[:, None] * Wv_).astype(f32))
    gg = _expand_cols((ln_g[:, None] * Wg).astype(f32))
    assert np.allclose(ln_b, 0.0), "ln_b folding not implemented"
    # bg folded into the tanh trick: sigmoid(z+bg) = .5*tanh((z+bg)/2)+.5
    bgx = np.zeros((128, NB), f32)
    for h in range(H):
        bgx[32 * (h % 3):32 * (h % 3) + DH, h // 3] = \
            bg[h * DH:(h + 1) * DH] / 2.0
    woe = _expand_rows(Wo.astype(f32)) * 0.5

    maskf = mask_b[0].astype(f32)  # [R, W]
    x_flat = x.reshape(R, W, DN)
    in_maps2 = []
    for c in range(NC):
        mrows = maskf[c * RPC:(c + 1) * RPC]  # [RPC, W]
        ngj = (mrows.reshape(RPC, 2, 128) - 1.0) * 1e38  # [r, jt, p]
        ngj = np.ascontiguousarray(
            ngj.transpose(2, 0, 1).reshape(128, RPC * 2))
        in_maps2.append({
            "x": np.ascontiguousarray(
                x_flat[c * RPC:(c + 1) * RPC].reshape(RPC * W, DN)),
            "wq": _bf16(gq), "wk": _bf16(gk), "wv": _bf16(gv),
            "wg": _bf16(gg), "wo": _bf16(woe),
            "bg": bgx, "bo": _bf16(bo.reshape(1, DN)),
            "bt": _bf16(bt), "ngj": ngj.astype(f32),
            "idm": _bf16(idm),
            "invm": (1.0 - mrows).astype(np.uint8),
        })
    return nc2, in_maps2


def kernel(**inputs):
    nc2, in_maps2 = _prep(**inputs)
    res2 = bass_utils.run_bass_kernel_spmd(nc2, in_maps2,
                                           core_ids=list(range(NC)),
                                           trace=TRACE)
    if TRACE:
        print("attn kernel exec_time_ns:", res2.exec_time_ns)
    out = np.concatenate(
        [res2.results[c]["o"].reshape(RPC, W, DN) for c in range(NC)],
        axis=0)
    return out.reshape(B, R, W, DN).astype(np.float32)
